# revision 29
# baseline (speedup 1.0000x reference)
"""Causal multi-head attention (QKV-packed) on 8 Trainium2 NeuronCores.

Sharding: pure head-parallel. B*H = 32 (batch, head) pairs -> 4 per core,
zero inter-core communication. Flash-style causal attention per head, all
in the "transposed" orientation (k on partitions) so no on-device
transposes are needed:

  - Host pre-lays-out Q^T, K^T as [D=128, S] (fp16, D on partitions) and V
    as k-blocks [128, D] (fp16). Scores for a PAIR of k-blocks land in one
    [128, 1024] PSUM tile (2 banks); one ACT instruction computes
    pt = exp(scale*s - 2) over the written extent. The -2 bias keeps
    exp <= ~45 < 240 (TRN fp8e4 max) and cancels between numerator and
    denominator. Diagonal blocks pack contiguously (t3 at [512,640), t2 at
    [512,768)) so no masked-garbage columns feed the denominator.
  - pt is fp8e4 except strip 0's diagonal pairs (bf16): short softmax rows
    (q < ~100) lack the num/den error cancellation that makes fp8 safe for
    long rows.
  - O^T[d, q] += V_j.T @ pt accumulates in PSUM per 512-col q-strip
    (fp16 x fp8/bf16, 1 col/cycle), then is evacuated to SBUF bf16 at
    strip end to free the bank.
  - Two heads are processed as a PAIR with strips interleaved
    (hA.s0, hB.s3, hA.s1, hB.s2, ...): each adjacent slot-pair has a
    constant amount of matmul work, so the PE never sees a multi-us idle
    stretch (which would re-engage the HAM clock throttle to 1.2 GHz).
  - Both heads' softmax denominators share ONE PSUM bank: strip s of the
    even head accumulates on partition 32s, of the odd head on 16+32s.
    The row is selected by the weight column of an M=128 matmul (fp8
    DoubleRow pairs at 2 cols/cycle for off-diagonal work; normal-rate
    ones-column matmuls for diagonal solo regions and strip 0). Weight
    columns that map to other live rows are exactly 0; never-live rows get
    2^-6 so their denominators stay finite (a 0 would turn the batched
    reciprocal into Inf and poison the K=32 broadcast matmul with 0*Inf).
    One DVE reciprocal per head pair serves all 8 strips.
  - Normalization: K=32 selector matmul broadcasts the reciprocal row into
    the retired den bank, one DVE cast to bf16, one bf16 DVE multiply.
    Output is bf16 (host casts to fp32). Epilogues of a head pair are
    spread one-per-slot across the next pair to avoid DVE pileups.
  - Zero-input warmup matmuls (no DMA dependency) run first so the PE HAM
    clock gate opens (1.2 -> 2.4 GHz) before real data arrives.
"""

import sys

if "/opt/trn_rl_repo" not in sys.path:
    sys.path.insert(0, "/opt/trn_rl_repo")

import numpy as np

B, S, H, D = 2, 2048, 16, 128
NCORES = 8
HPC = (B * H) // NCORES  # heads per core = 4
QS = 512   # q-strip width (PSUM bank)
KB = 128   # k-block (partition dim)
NEG = -1.0e30
SCALE = 1.0 / float(np.sqrt(D))
EXP_BIAS = -2.0
NSTRIP = S // QS  # 4
EPS8 = 0.015625  # 2^-6, min normal e4m3

_nc_cache = {}


def _build_nc():
    import concourse.bass as bass  # noqa: F401
    import concourse.mybir as mybir
    from concourse import bacc
    from concourse.tile import TileContext

    f32 = mybir.dt.float32
    f16 = mybir.dt.float16
    f8 = mybir.dt.float8e4
    bf16 = mybir.dt.bfloat16
    f32r = mybir.dt.float32r
    Exp = mybir.ActivationFunctionType.Exp
    DR = mybir.MatmulPerfMode.DoubleRow

    nc = bacc.Bacc()
    # One packed input per head [128, 3*S] fp16:
    # cols [0,S) = Q^T, [S,2S) = K^T, [2S,3S) = V swizzled so column
    # block j holds the V k-block [128, D] (v[p, j*KB+d] = V[j*KB+p, d]).
    qkvT = nc.declare_dram_parameter("qkvT", [HPC, 128, 3 * S], f16, isOutput=False)
    v8_d = nc.declare_dram_parameter("v8", [HPC, 128, 1536], f8, isOutput=False)
    tri_d = nc.declare_dram_parameter("tri", [128, 128], f32, isOutput=False)
    ones8_d = nc.declare_dram_parameter("ones8", [128, 512], f8, isOutput=False)
    selv_d = nc.declare_dram_parameter("selv", [128, 256], f32r, isOutput=False)
    oT = nc.declare_dram_parameter("oT", [HPC, 128, S], bf16, isOutput=True)

    with TileContext(nc) as tc:
        with (
            nc.allow_low_precision(reason="fp16/fp8/bf16 staging is within tolerance"),
            tc.tile_pool(name="cpool", bufs=1) as cpool,
            tc.tile_pool(name="qkpool", bufs=4) as qkpool,
            tc.tile_pool(name="ptpool", bufs=8) as ptpool,
            tc.tile_pool(name="ptbpool", bufs=4) as ptbpool,
            tc.tile_pool(name="orpool", bufs=16) as orpool,
            tc.tile_pool(name="rcpool", bufs=2) as rcpool,
            tc.tile_pool(name="obpool", bufs=4) as obpool,
            tc.tile_pool(name="scp", bufs=2, space="PSUM") as scp,
            tc.tile_pool(name="pso", bufs=2, space="PSUM") as pso,
            tc.tile_pool(name="psd", bufs=2, space="PSUM") as psd,
        ):
            tri_sb = cpool.tile([128, 128], f32)
            nc.sync.dma_start(out=tri_sb[:], in_=tri_d[:])
            ones8 = cpool.tile([128, 512], f8)
            nc.sync.dma_start(out=ones8[:], in_=ones8_d[:])
            selv = cpool.tile([128, 256], f32r)
            nc.sync.dma_start(out=selv[:], in_=selv_d[:])
            biasc = cpool.tile([128, 1], f32)
            nc.gpsimd.memset(biasc[:], EXP_BIAS)
            # [128, 2, 256] pair view of the den weights (1.0 at m=112)
            o83 = ones8[:, 0:512].rearrange("p (a m) -> p a m", a=2)

            # HAM warmup: zero-input matmuls with no DMA dependency keep the
            # PE busy from the end of the preamble so the clock gate opens
            # (1.2 -> 2.4 GHz) before the first real matmul.
            zsrc = cpool.tile([128, 384], f16)
            nc.scalar.memzero(zsrc[:])
            wps = scp.tile([128, 1024], f32, tag="sc")
            for w in range(40):
                nc.tensor.matmul(
                    wps[:, 0:256],
                    lhsT=zsrc[:, 0:128],
                    rhs=zsrc[:, 128:384],
                    start=True,
                    stop=True,
                )

            def emit_epilogue(ep, tail=False):
                h, s, row, den_t, recip_t = ep
                o_raw = o_raw_of[(h, s)]
                # broadcast recip row across partitions via a K=32 selector
                # matmul; mid-kernel it lands in the retired den bank, at
                # the kernel tail in alternating (free) score-pool banks so
                # consecutive epilogues don't serialize on one bank
                wa = 32 * s
                sel = selv[wa : wa + 32, 0:128] if row % 32 == 0 else (
                    selv[wa : wa + 32, 128:256]
                )
                if tail:
                    rbt = scp.tile([128, 1024], f32, tag="sc")
                    rb = rbt[:, 0:QS]
                else:
                    rb = den_t[:, :]
                nc.tensor.matmul(
                    rb,
                    lhsT=sel,
                    rhs=recip_t[wa : wa + 32, 0:QS],
                    start=True,
                    stop=True,
                    tile_position=(wa, 0),
                )
                o_sb = obpool.tile([128, QS], bf16, tag="o_sb")
                nc.vector.tensor_mul(o_sb[:], o_raw[:], rb)
                nc.sync.dma_start(out=oT[h][:, QS * s : QS * (s + 1)], in_=o_sb[:])

            pending = []
            o_raw_of = {}
            prefetched = {}

            def load_pair(hA, hB):
                if hA in prefetched:
                    return prefetched[hA]
                sbufs = {}
                for h in (hA, hB):
                    qkv_sb = qkpool.tile([128, 3 * S], f16, tag="qkv_sb")
                    sbufs[h] = qkv_sb
                if hA == 0:
                    # interleave both heads' critical chunks so slot 1
                    # (hA strip 0) and slot 2 (hB strip 3) both start early
                    for h, c0, c1 in (
                        (hA, S, S + 512),          # hA K^T blocks 0-3
                        (hA, 0, 512),              # hA Q^T strip 0
                        (hA, 2 * S, 2 * S + 512),  # hA V blocks 0-3
                        (hB, S, 2 * S),            # hB K^T
                        (hB, 3 * QS, S),           # hB Q^T strip 3
                        (hB, 2 * S, 3 * S),        # hB V
                        (hA, 512, S),              # hA Q^T rest
                        (hA, S + 512, 2 * S),      # hA K^T rest
                        (hA, 2 * S + 512, 3 * S),  # hA V rest
                        (hB, 0, 3 * QS),           # hB Q^T rest
                    ):
                        nc.sync.dma_start(
                            out=sbufs[h][:, c0:c1], in_=qkvT[h][:, c0:c1]
                        )
                else:
                    for h in (hA, hB):
                        nc.sync.dma_start(out=sbufs[h][:], in_=qkvT[h])
                v8s = {}
                for h in (hA, hB):
                    v8_sb = qkpool.tile([128, 1536], f8, tag="v8_sb")
                    nc.sync.dma_start(out=v8_sb[:], in_=v8_d[h])
                    v8s[h] = v8_sb
                prefetched[hA] = (sbufs, v8s)
                return prefetched[hA]

            def emit_strip(h, s, qkv_sb, v8_sb, den, row, den_state, pair_last_slot):
                """Emit one q-strip of head h. den_state = [started]."""
                qt = qkv_sb[:, 0:S]
                kt = qkv_sb[:, S : 2 * S]
                vv = qkv_sb[:, 2 * S : 3 * S]
                r = row
                o_ps = pso.tile([128, QS], f32, tag="o_ps")
                q0 = QS * s

                def den_flags(last_of_strip):
                    st = not den_state[0]
                    den_state[0] = True
                    sp = pair_last_slot and last_of_strip
                    return st, sp

                # pairs: (jA, jB, woff, wN, bcol, NB, triA, triB)
                pairs = []
                for p in range(2 * s):
                    pairs.append((2 * p, 2 * p + 1, 0, QS, 512, QS, None, None))
                t0, t1, t2, t3 = 4 * s, 4 * s + 1, 4 * s + 2, 4 * s + 3
                # X = (t0, t3): t0 covers [0,512), t3 at tile [512,640)
                pairs.append((t0, t3, 0, QS, 512, 128, 0, 384))
                # Y = (t1, t2): t1 covers [128,512) at tile [0,384),
                # t2 at tile [512,768); tile [384,512) stays stale and its
                # exp output is never read
                pairs.append((t1, t2, 128, 384, 512, 256, 128, 256))

                npair = len(pairs)
                deferred = []

                def emit_pv_den(p):
                    (pi, jA, jB, woff, wN, bcol, NB, pt, diag, s0d) = p
                    first, last = (pi == 0), (pi == npair - 1)
                    if not diag:
                        # full pair: one fp8 DoubleRow PV over both k-blocks
                        # (V fp8 is safe off-diagonal: long-row softmax is
                        # diffuse, so per-element V quantization noise
                        # averages out)
                        v8p = v8_sb[:, 256 * (jA // 2) : 256 * (jA // 2) + 256]
                        nc.tensor.matmul(
                            o_ps[:, 0:QS],
                            lhsT=v8p.rearrange("p (a d) -> p a d", a=2),
                            rhs=pt[:, 0:1024].rearrange("p (a b) -> p a b", a=2),
                            start=first,
                            stop=False,
                            perf_mode=DR,
                        )
                    else:
                        nc.tensor.matmul(
                            o_ps[:, woff : woff + wN],
                            lhsT=vv[:, KB * jA : KB * (jA + 1)],
                            rhs=pt[:, 0:wN],
                            start=first,
                            stop=False,
                        )
                        nc.tensor.matmul(
                            o_ps[:, woff + wN - NB : woff + wN],
                            lhsT=vv[:, KB * jB : KB * (jB + 1)],
                            rhs=pt[:, bcol : bcol + NB],
                            start=False,
                            stop=last,
                        )
                    if not diag:
                        # full pair: DoubleRow over the whole strip
                        st, sp = den_flags(False)
                        nc.tensor.matmul(
                            den[0:128, 0:QS],
                            lhsT=o83[:, :, 112 - r : 240 - r],
                            rhs=pt[:, 0:1024].rearrange("p (a b) -> p a b", a=2),
                            start=st,
                            stop=sp,
                            perf_mode=DR,
                        )
                    elif s0d:
                        # strip 0 diagonal (bf16 pt): two normal-rate
                        # ones-column matmuls per pair
                        st, _ = den_flags(False)
                        nc.tensor.matmul(
                            den[0:128, woff : woff + wN],
                            lhsT=ones8[:, 112 - r : 240 - r],
                            rhs=pt[:, 0:wN],
                            start=st,
                            stop=False,
                        )
                        st, sp = den_flags(last)
                        nc.tensor.matmul(
                            den[0:128, woff + wN - NB : woff + wN],
                            lhsT=ones8[:, 112 - r : 240 - r],
                            rhs=pt[:, bcol : bcol + NB],
                            start=False,
                            stop=sp,
                        )
                    else:
                        # diagonal pair, fp8: solo region (A only) at normal
                        # rate + the overlap region as a DoubleRow pair with
                        # stride (bcol - solo)
                        solo = wN - NB
                        st, _ = den_flags(False)
                        nc.tensor.matmul(
                            den[0:128, woff : woff + solo],
                            lhsT=ones8[:, 112 - r : 240 - r],
                            rhs=pt[:, 0:solo],
                            start=st,
                            stop=False,
                        )
                        st, sp = den_flags(last)
                        pr = pt[:, solo : solo + 2 * (bcol - solo)].rearrange(
                            "p (a b) -> p a b", a=2
                        )
                        nc.tensor.matmul(
                            den[0:128, woff + solo : woff + wN],
                            lhsT=o83[:, :, 112 - r : 240 - r],
                            rhs=pr[:, :, 0:NB],
                            start=False,
                            stop=sp,
                            perf_mode=DR,
                        )

                for pi, (jA, jB, woff, wN, bcol, NB, trA, trB) in enumerate(pairs):
                    diag = pi >= npair - 2
                    s0d = diag and s == 0
                    sc = scp.tile([128, 1024], f32, tag="sc")
                    nc.tensor.matmul(
                        sc[:, 0:wN],
                        lhsT=kt[:, KB * jA : KB * (jA + 1)],
                        rhs=qt[:, q0 + woff : q0 + woff + wN],
                        start=True,
                        stop=True,
                    )
                    bq = q0 + woff + wN - NB
                    nc.tensor.matmul(
                        sc[:, bcol : bcol + NB],
                        lhsT=kt[:, KB * jB : KB * (jB + 1)],
                        rhs=qt[:, bq : bq + NB],
                        start=True,
                        stop=True,
                    )
                    if trA is not None:
                        # both 128-col triangular masks in one strided DVE op
                        sc3 = sc[:, 0:1024].rearrange("p (a b) -> p a b", a=2)[
                            :, :, 0:128
                        ]
                        tri3 = tri_sb[:, 0:128].unsqueeze(1).broadcast_to(
                            [128, 2, 128]
                        )
                        nc.vector.tensor_add(sc3, sc3, tri3)
                    ext = bcol + NB
                    if s0d:
                        pt = ptbpool.tile([128, 1024], bf16, tag="ptb")
                    else:
                        pt = ptpool.tile([128, 1024], f8, tag="pt")
                    nc.scalar.activation(
                        pt[:, 0:ext], sc[:, 0:ext], Exp, bias=biasc[:], scale=SCALE
                    )
                    # defer PV/den two pairs so the PE always has score
                    # matmuls queued ahead of work that waits on ACT output
                    deferred.append((pi, jA, jB, woff, wN, bcol, NB, pt, diag, s0d))
                    if len(deferred) > 3:
                        emit_pv_den(deferred.pop(0))
                while deferred:
                    emit_pv_den(deferred.pop(0))
                # evacuate O^T early (frees the PSUM bank; bf16 is fine for
                # the un-normalized accumulator)
                o_raw = orpool.tile([128, QS], bf16, tag="o_raw")
                nc.vector.tensor_copy(o_raw[:], o_ps[:])
                o_raw_of[(h, s)] = o_raw

            for pr_i in range(HPC // 2):
                hA, hB = 2 * pr_i, 2 * pr_i + 1
                sbufs = {}
                for h in (hA, hB):
                    qkv_sb = qkpool.tile([128, 3 * S], f16, tag="qkv_sb")
                    sbufs[h] = qkv_sb
                if hA == 0:
                    # interleave both heads' critical chunks so slot 1
                    # (hA strip 0) and slot 2 (hB strip 3) both start early
                    for h, c0, c1 in (
                        (hA, S, S + 512),          # hA K^T blocks 0-3
                        (hA, 0, 512),              # hA Q^T strip 0
                        (hA, 2 * S, 2 * S + 512),  # hA V blocks 0-3
                        (hB, S, 2 * S),            # hB K^T
                        (hB, 3 * QS, S),           # hB Q^T strip 3
                        (hB, 2 * S, 3 * S),        # hB V
                        (hA, 512, S),              # hA Q^T rest
                        (hA, S + 512, 2 * S),      # hA K^T rest
                        (hA, 2 * S + 512, 3 * S),  # hA V rest
                        (hB, 0, 3 * QS),           # hB Q^T rest
                    ):
                        nc.sync.dma_start(
                            out=sbufs[h][:, c0:c1], in_=qkvT[h][:, c0:c1]
                        )
                else:
                    for h in (hA, hB):
                        nc.sync.dma_start(out=sbufs[h][:], in_=qkvT[h])

                v8s = {}
                for h in (hA, hB):
                    v8_sb = qkpool.tile([128, 1536], f8, tag="v8_sb")
                    nc.sync.dma_start(out=v8_sb[:], in_=v8_d[h])
                    v8s[h] = v8_sb
                den = psd.tile([128, QS], f32, tag="den")
                den_state = [False]
                slots = []
                for s in range(NSTRIP):
                    slots.append((hA, s))
                    slots.append((hB, NSTRIP - 1 - s))
                for si, (h, s) in enumerate(slots):
                    row = 32 * s + (16 if h % 2 else 0)
                    emit_strip(
                        h, s, sbufs[h], v8s[h], den, row, den_state,
                        si == len(slots) - 1,
                    )
                    if si == 2 and pr_i + 1 < HPC // 2:
                        # prefetch the next pair's inputs mid-pair so its
                        # first slots never wait on DMA
                        load_pair(2 * pr_i + 2, 2 * pr_i + 3)
                    if pending:
                        emit_epilogue(pending.pop(0))

                # one batched reciprocal per head pair (rows 0,16,...,112
                # hold the 8 strips' denominators)
                recip = rcpool.tile([128, QS], f32r, tag="recip")
                nc.vector.reciprocal(recip[:], den[:])
                for h in (hA, hB):
                    for s in range(NSTRIP):
                        row = 32 * s + (16 if h % 2 else 0)
                        pending.append((h, s, row, den, recip))
            while pending:
                emit_epilogue(pending.pop(0), tail=True)
    nc.compile()
    return nc


def get_nc():
    if "nc" not in _nc_cache:
        _nc_cache["nc"] = _build_nc()
    return _nc_cache["nc"]


def _build_tri():
    dk = np.arange(128)[:, None]
    c = np.arange(128)[None, :]
    return np.where(dk <= c, 0.0, NEG).astype(np.float32)


def make_in_maps(qkv):
    import ml_dtypes

    qkv = np.asarray(qkv, dtype=np.float32)
    tri = _build_tri()
    # Denominator weights, [128, (a=2) x (m=256)] fp8: column m selects the
    # output partition of an M=128 matmul sliced at [112-r, 240-r). 1.0 at
    # m=112 (the target row r); exactly 0 at other m = 0 mod 16 (those map
    # onto other LIVE den rows); 2^-6 elsewhere so never-live rows hold a
    # finite denominator (reciprocal of 0 would be Inf, and 0*Inf = NaN in
    # the K=32 broadcast matmul).
    m = np.arange(256)
    col = np.where(m % 16 == 0, 0.0, EPS8).astype(np.float32)
    col[112] = 1.0
    ones8 = np.broadcast_to(
        np.concatenate([col, col])[None, :], (128, 512)
    ).astype(ml_dtypes.float8_e4m3)
    # K=32 selector weights for the reciprocal broadcast: partition p of
    # cols [0,128) is 1.0 iff p % 32 == 0; of cols [128,256) iff p % 32 == 16.
    p = np.arange(128)[:, None]
    selv = np.concatenate(
        [
            np.where(p % 32 == 0, 1.0, 0.0).repeat(128, axis=1),
            np.where(p % 32 == 16, 1.0, 0.0).repeat(128, axis=1),
        ],
        axis=1,
    ).astype(np.float32)
    in_maps = []
    for core in range(NCORES):
        qkvT = np.empty((HPC, 128, 3 * S), np.float16)
        for i in range(HPC):
            bh = core * HPC + i
            b, h = bh // H, bh % H
            qkvT[i, :, 0:S] = qkv[b, :, 0, h, :].T
            qkvT[i, :, S : 2 * S] = qkv[b, :, 1, h, :].T
            qkvT[i, :, 2 * S : 3 * S] = (
                qkv[b, :, 2, h, :]
                .reshape(S // KB, KB, D)
                .transpose(1, 0, 2)
                .reshape(KB, S)
            )
        v8 = np.empty((HPC, 128, 1536), ml_dtypes.float8_e4m3)
        for i in range(HPC):
            bh = core * HPC + i
            b, h = bh // H, bh % H
            vkd = qkv[b, :, 2, h, :].reshape(S // KB, KB, D)  # [16, 128, 128]
            # pair p covers blocks (2p, 2p+1); layout [128, (a=2) x (d=128)]
            v8[i] = (
                vkd[0:12]
                .reshape(6, 2, KB, D)
                .transpose(2, 0, 1, 3)
                .reshape(KB, 1536)
            )
        in_maps.append(
            {"qkvT": qkvT, "tri": tri, "ones8": ones8, "selv": selv, "v8": v8}
        )
    return in_maps


def assemble_out(results):
    out = np.empty((B, S, H, D), np.float32)
    for core in range(NCORES):
        oTc = results[core]["oT"]  # [HPC, 128, S] bf16
        for i in range(HPC):
            bh = core * HPC + i
            b, h = bh // H, bh % H
            out[b, :, h, :] = oTc[i].astype(np.float32).T
    return out


def kernel(qkv):
    from concourse.bass_utils import run_bass_kernel_spmd

    in_maps = make_in_maps(qkv)
    nc = get_nc()
    res = run_bass_kernel_spmd(nc, in_maps, list(range(NCORES)))
    return assemble_out(res.results)


# revision 30
# speedup vs baseline: 1.1747x; 1.1747x over previous
"""Causal multi-head attention (QKV-packed) on 8 Trainium2 NeuronCores.

Sharding: pure head-parallel. B*H = 32 (batch, head) pairs -> 4 per core,
zero inter-core communication. Flash-style causal attention per head, all
in the "transposed" orientation (k on partitions) so no on-device
transposes are needed:

  - Host pre-lays-out Q^T, K^T as [D=128, S] (fp16, D on partitions) and V
    as k-blocks [128, D] (fp16). Scores for a PAIR of k-blocks land in one
    [128, 1024] PSUM tile (2 banks); one ACT instruction computes
    pt = exp(scale*s - 2) over the written extent. The -2 bias keeps
    exp <= ~45 < 240 (TRN fp8e4 max) and cancels between numerator and
    denominator. Diagonal blocks pack contiguously (t3 at [512,640), t2 at
    [512,768)) so no masked-garbage columns feed the denominator.
  - pt is fp8e4 except strip 0's diagonal pairs (bf16): short softmax rows
    (q < ~100) lack the num/den error cancellation that makes fp8 safe for
    long rows.
  - O^T[d, q] += V_j.T @ pt accumulates in PSUM per 512-col q-strip
    (fp16 x fp8/bf16, 1 col/cycle), then is evacuated to SBUF bf16 at
    strip end to free the bank.
  - Two heads are processed as a PAIR with strips interleaved
    (hA.s0, hB.s3, hA.s1, hB.s2, ...): each adjacent slot-pair has a
    constant amount of matmul work, so the PE never sees a multi-us idle
    stretch (which would re-engage the HAM clock throttle to 1.2 GHz).
  - Both heads' softmax denominators share ONE PSUM bank: strip s of the
    even head accumulates on partition 32s, of the odd head on 16+32s.
    The row is selected by the weight column of an M=128 matmul (fp8
    DoubleRow pairs at 2 cols/cycle for off-diagonal work; normal-rate
    ones-column matmuls for diagonal solo regions and strip 0). Weight
    columns that map to other live rows are exactly 0; never-live rows get
    2^-6 so their denominators stay finite (a 0 would turn the batched
    reciprocal into Inf and poison the K=32 broadcast matmul with 0*Inf).
    One DVE reciprocal per head pair serves all 8 strips.
  - Normalization: K=32 selector matmul broadcasts the reciprocal row into
    the retired den bank, one DVE cast to bf16, one bf16 DVE multiply.
    Output is bf16 (host casts to fp32). Epilogues of a head pair are
    spread one-per-slot across the next pair to avoid DVE pileups.
  - Zero-input warmup matmuls (no DMA dependency) run first so the PE HAM
    clock gate opens (1.2 -> 2.4 GHz) before real data arrives.
"""

import sys

if "/opt/trn_rl_repo" not in sys.path:
    sys.path.insert(0, "/opt/trn_rl_repo")

import numpy as np

B, S, H, D = 2, 2048, 16, 128
NCORES = 8
HPC = (B * H) // NCORES  # heads per core = 4
QS = 512   # q-strip width (PSUM bank)
KB = 128   # k-block (partition dim)
NEG = -1.0e30
SCALE = 1.0 / float(np.sqrt(D))
EXP_BIAS = -2.0
NSTRIP = S // QS  # 4
EPS8 = 0.015625  # 2^-6, min normal e4m3

_nc_cache = {}


def _build_nc():
    import concourse.bass as bass  # noqa: F401
    import concourse.mybir as mybir
    from concourse import bacc
    from concourse.tile import TileContext

    f32 = mybir.dt.float32
    f16 = mybir.dt.float16
    f8 = mybir.dt.float8e4
    bf16 = mybir.dt.bfloat16
    f32r = mybir.dt.float32r
    Exp = mybir.ActivationFunctionType.Exp
    DR = mybir.MatmulPerfMode.DoubleRow

    nc = bacc.Bacc()
    # One packed input per head [128, 3*S] fp16:
    # cols [0,S) = Q^T, [S,2S) = K^T, [2S,3S) = V swizzled so column
    # block j holds the V k-block [128, D] (v[p, j*KB+d] = V[j*KB+p, d]).
    qkvT = nc.declare_dram_parameter("qkvT", [HPC, 128, 3 * S], f16, isOutput=False)
    v8_d = nc.declare_dram_parameter("v8", [HPC, 128, 1536], f8, isOutput=False)
    tri_d = nc.declare_dram_parameter("tri", [128, 128], f32, isOutput=False)
    ones8_d = nc.declare_dram_parameter("ones8", [128, 512], f8, isOutput=False)
    selv_d = nc.declare_dram_parameter("selv", [128, 256], f32r, isOutput=False)
    oT = nc.declare_dram_parameter("oT", [HPC, 128, S], bf16, isOutput=True)

    with TileContext(nc) as tc:
        with (
            nc.allow_low_precision(reason="fp16/fp8/bf16 staging is within tolerance"),
            tc.tile_pool(name="cpool", bufs=1) as cpool,
            tc.tile_pool(name="qkpool", bufs=4) as qkpool,
            tc.tile_pool(name="ptpool", bufs=8) as ptpool,
            tc.tile_pool(name="ptbpool", bufs=4) as ptbpool,
            tc.tile_pool(name="orpool", bufs=16) as orpool,
            tc.tile_pool(name="rcpool", bufs=2) as rcpool,
            tc.tile_pool(name="obpool", bufs=4) as obpool,
            tc.tile_pool(name="scp", bufs=2, space="PSUM") as scp,
            tc.tile_pool(name="pso", bufs=2, space="PSUM") as pso,
            tc.tile_pool(name="psd", bufs=2, space="PSUM") as psd,
        ):
            tri_sb = cpool.tile([128, 128], f32)
            nc.sync.dma_start(out=tri_sb[:], in_=tri_d[:])
            ones8 = cpool.tile([128, 512], f8)
            nc.sync.dma_start(out=ones8[:], in_=ones8_d[:])
            selv = cpool.tile([128, 256], f32r)
            nc.sync.dma_start(out=selv[:], in_=selv_d[:])
            biasc = cpool.tile([128, 1], f32)
            nc.gpsimd.memset(biasc[:], EXP_BIAS)
            # [128, 2, 256] pair view of the den weights (1.0 at m=112)
            o83 = ones8[:, 0:512].rearrange("p (a m) -> p a m", a=2)

            # HAM warmup: zero-input matmuls with no DMA dependency keep the
            # PE busy from the end of the preamble so the clock gate opens
            # (1.2 -> 2.4 GHz) before the first real matmul.
            zsrc = cpool.tile([128, 384], f16)
            nc.scalar.memzero(zsrc[:])
            wps = scp.tile([128, 1024], f32, tag="sc")
            for w in range(40):
                nc.tensor.matmul(
                    wps[:, 0:256],
                    lhsT=zsrc[:, 0:128],
                    rhs=zsrc[:, 128:384],
                    start=True,
                    stop=True,
                )

            def emit_epilogue(ep, tail=False):
                h, s, row, den_t, recip_t = ep
                o_raw = o_raw_of[(h, s)]
                # broadcast recip row across partitions via a K=32 selector
                # matmul; mid-kernel it lands in the retired den bank, at
                # the kernel tail in alternating (free) score-pool banks so
                # consecutive epilogues don't serialize on one bank
                wa = 32 * s
                sel = selv[wa : wa + 32, 0:128] if row % 32 == 0 else (
                    selv[wa : wa + 32, 128:256]
                )
                if tail:
                    rbt = scp.tile([128, 1024], f32, tag="sc")
                    rb = rbt[:, 0:QS]
                else:
                    rb = den_t[:, :]
                nc.tensor.matmul(
                    rb,
                    lhsT=sel,
                    rhs=recip_t[wa : wa + 32, 0:QS],
                    start=True,
                    stop=True,
                    tile_position=(wa, 0),
                )
                o_sb = obpool.tile([128, QS], bf16, tag="o_sb")
                nc.vector.tensor_mul(o_sb[:], o_raw[:], rb)
                nc.sync.dma_start(out=oT[h][:, QS * s : QS * (s + 1)], in_=o_sb[:])

            pending = []
            o_raw_of = {}
            prefetched = {}

            def load_pair(hA, hB):
                if hA in prefetched:
                    return prefetched[hA]
                sbufs = {}
                for h in (hA, hB):
                    qkv_sb = qkpool.tile([128, 3 * S], f16, tag="qkv_sb")
                    sbufs[h] = qkv_sb
                if hA == 0:
                    # interleave both heads' critical chunks so slot 1
                    # (hA strip 0) and slot 2 (hB strip 3) both start early
                    for h, c0, c1 in (
                        (hA, S, S + 512),          # hA K^T blocks 0-3
                        (hA, 0, 512),              # hA Q^T strip 0
                        (hA, 2 * S, 2 * S + 512),  # hA V blocks 0-3
                        (hB, S, 2 * S),            # hB K^T
                        (hB, 3 * QS, S),           # hB Q^T strip 3
                        (hB, 2 * S, 3 * S),        # hB V
                        (hA, 512, S),              # hA Q^T rest
                        (hA, S + 512, 2 * S),      # hA K^T rest
                        (hA, 2 * S + 512, 3 * S),  # hA V rest
                        (hB, 0, 3 * QS),           # hB Q^T rest
                    ):
                        nc.sync.dma_start(
                            out=sbufs[h][:, c0:c1], in_=qkvT[h][:, c0:c1]
                        )
                else:
                    for h, c0, c1 in (
                        (hA, S, S + 512),          # hA K^T blocks 0-3
                        (hA, 0, 512),              # hA Q^T strip 0
                        (hA, 2 * S, 2 * S + 512),  # hA V blocks 0-3
                        (hB, S, 2 * S),            # hB K^T
                        (hB, 3 * QS, S),           # hB Q^T strip 3
                        (hB, 2 * S, 3 * S),        # hB V
                        (hA, 512, S),              # hA Q^T rest
                        (hA, S + 512, 2 * S),      # hA K^T rest
                        (hA, 2 * S + 512, 3 * S),  # hA V rest
                        (hB, 0, 3 * QS),           # hB Q^T rest
                    ):
                        nc.sync.dma_start(
                            out=sbufs[h][:, c0:c1], in_=qkvT[h][:, c0:c1]
                        )
                v8s = {}
                for h in (hA, hB):
                    v8_sb = qkpool.tile([128, 1536], f8, tag="v8_sb")
                    nc.sync.dma_start(out=v8_sb[:], in_=v8_d[h])
                    v8s[h] = v8_sb
                prefetched[hA] = (sbufs, v8s)
                return prefetched[hA]

            def emit_strip(h, s, qkv_sb, v8_sb, den, row, den_state, pair_last_slot):
                """Emit one q-strip of head h. den_state = [started]."""
                qt = qkv_sb[:, 0:S]
                kt = qkv_sb[:, S : 2 * S]
                vv = qkv_sb[:, 2 * S : 3 * S]
                r = row
                o_ps = pso.tile([128, QS], f32, tag="o_ps")
                q0 = QS * s

                def den_flags(last_of_strip):
                    st = not den_state[0]
                    den_state[0] = True
                    sp = pair_last_slot and last_of_strip
                    return st, sp

                # pairs: (jA, jB, woff, wN, bcol, NB, triA, triB)
                pairs = []
                for p in range(2 * s):
                    pairs.append((2 * p, 2 * p + 1, 0, QS, 512, QS, None, None))
                t0, t1, t2, t3 = 4 * s, 4 * s + 1, 4 * s + 2, 4 * s + 3
                # X = (t0, t3): t0 covers [0,512), t3 at tile [512,640)
                pairs.append((t0, t3, 0, QS, 512, 128, 0, 384))
                # Y = (t1, t2): t1 covers [128,512) at tile [0,384),
                # t2 at tile [512,768); tile [384,512) stays stale and its
                # exp output is never read
                pairs.append((t1, t2, 128, 384, 512, 256, 128, 256))

                npair = len(pairs)
                deferred = []

                def emit_pv_den(p):
                    (pi, jA, jB, woff, wN, bcol, NB, pt, diag, s0d) = p
                    first, last = (pi == 0), (pi == npair - 1)
                    if not diag:
                        # full pair: one fp8 DoubleRow PV over both k-blocks
                        # (V fp8 is safe off-diagonal: long-row softmax is
                        # diffuse, so per-element V quantization noise
                        # averages out)
                        v8p = v8_sb[:, 256 * (jA // 2) : 256 * (jA // 2) + 256]
                        nc.tensor.matmul(
                            o_ps[:, 0:QS],
                            lhsT=v8p.rearrange("p (a d) -> p a d", a=2),
                            rhs=pt[:, 0:1024].rearrange("p (a b) -> p a b", a=2),
                            start=first,
                            stop=False,
                            perf_mode=DR,
                        )
                    else:
                        nc.tensor.matmul(
                            o_ps[:, woff : woff + wN],
                            lhsT=vv[:, KB * jA : KB * (jA + 1)],
                            rhs=pt[:, 0:wN],
                            start=first,
                            stop=False,
                        )
                        nc.tensor.matmul(
                            o_ps[:, woff + wN - NB : woff + wN],
                            lhsT=vv[:, KB * jB : KB * (jB + 1)],
                            rhs=pt[:, bcol : bcol + NB],
                            start=False,
                            stop=last,
                        )
                    if not diag:
                        # full pair: DoubleRow over the whole strip
                        st, sp = den_flags(False)
                        nc.tensor.matmul(
                            den[0:128, 0:QS],
                            lhsT=o83[:, :, 112 - r : 240 - r],
                            rhs=pt[:, 0:1024].rearrange("p (a b) -> p a b", a=2),
                            start=st,
                            stop=sp,
                            perf_mode=DR,
                        )
                    elif s0d:
                        # strip 0 diagonal (bf16 pt): two normal-rate
                        # ones-column matmuls per pair
                        st, _ = den_flags(False)
                        nc.tensor.matmul(
                            den[0:128, woff : woff + wN],
                            lhsT=ones8[:, 112 - r : 240 - r],
                            rhs=pt[:, 0:wN],
                            start=st,
                            stop=False,
                        )
                        st, sp = den_flags(last)
                        nc.tensor.matmul(
                            den[0:128, woff + wN - NB : woff + wN],
                            lhsT=ones8[:, 112 - r : 240 - r],
                            rhs=pt[:, bcol : bcol + NB],
                            start=False,
                            stop=sp,
                        )
                    else:
                        # diagonal pair, fp8: solo region (A only) at normal
                        # rate + the overlap region as a DoubleRow pair with
                        # stride (bcol - solo)
                        solo = wN - NB
                        st, _ = den_flags(False)
                        nc.tensor.matmul(
                            den[0:128, woff : woff + solo],
                            lhsT=ones8[:, 112 - r : 240 - r],
                            rhs=pt[:, 0:solo],
                            start=st,
                            stop=False,
                        )
                        st, sp = den_flags(last)
                        pr = pt[:, solo : solo + 2 * (bcol - solo)].rearrange(
                            "p (a b) -> p a b", a=2
                        )
                        nc.tensor.matmul(
                            den[0:128, woff + solo : woff + wN],
                            lhsT=o83[:, :, 112 - r : 240 - r],
                            rhs=pr[:, :, 0:NB],
                            start=False,
                            stop=sp,
                            perf_mode=DR,
                        )

                for pi, (jA, jB, woff, wN, bcol, NB, trA, trB) in enumerate(pairs):
                    diag = pi >= npair - 2
                    s0d = diag and s == 0
                    sc = scp.tile([128, 1024], f32, tag="sc")
                    nc.tensor.matmul(
                        sc[:, 0:wN],
                        lhsT=kt[:, KB * jA : KB * (jA + 1)],
                        rhs=qt[:, q0 + woff : q0 + woff + wN],
                        start=True,
                        stop=True,
                    )
                    bq = q0 + woff + wN - NB
                    nc.tensor.matmul(
                        sc[:, bcol : bcol + NB],
                        lhsT=kt[:, KB * jB : KB * (jB + 1)],
                        rhs=qt[:, bq : bq + NB],
                        start=True,
                        stop=True,
                    )
                    if trA is not None:
                        # both 128-col triangular masks in one strided DVE op
                        sc3 = sc[:, 0:1024].rearrange("p (a b) -> p a b", a=2)[
                            :, :, 0:128
                        ]
                        tri3 = tri_sb[:, 0:128].unsqueeze(1).broadcast_to(
                            [128, 2, 128]
                        )
                        nc.vector.tensor_add(sc3, sc3, tri3)
                    ext = bcol + NB
                    if s0d:
                        pt = ptbpool.tile([128, 1024], bf16, tag="ptb")
                    else:
                        pt = ptpool.tile([128, 1024], f8, tag="pt")
                    nc.scalar.activation(
                        pt[:, 0:ext], sc[:, 0:ext], Exp, bias=biasc[:], scale=SCALE
                    )
                    # defer PV/den two pairs so the PE always has score
                    # matmuls queued ahead of work that waits on ACT output
                    deferred.append((pi, jA, jB, woff, wN, bcol, NB, pt, diag, s0d))
                    if len(deferred) > 3:
                        emit_pv_den(deferred.pop(0))
                while deferred:
                    emit_pv_den(deferred.pop(0))
                # evacuate O^T early (frees the PSUM bank; bf16 is fine for
                # the un-normalized accumulator)
                o_raw = orpool.tile([128, QS], bf16, tag="o_raw")
                nc.vector.tensor_copy(o_raw[:], o_ps[:])
                o_raw_of[(h, s)] = o_raw

            for pr_i in range(HPC // 2):
                hA, hB = 2 * pr_i, 2 * pr_i + 1
                sbufs = {}
                for h in (hA, hB):
                    qkv_sb = qkpool.tile([128, 3 * S], f16, tag="qkv_sb")
                    sbufs[h] = qkv_sb
                if hA == 0:
                    # interleave both heads' critical chunks so slot 1
                    # (hA strip 0) and slot 2 (hB strip 3) both start early
                    for h, c0, c1 in (
                        (hA, S, S + 512),          # hA K^T blocks 0-3
                        (hA, 0, 512),              # hA Q^T strip 0
                        (hA, 2 * S, 2 * S + 512),  # hA V blocks 0-3
                        (hB, S, 2 * S),            # hB K^T
                        (hB, 3 * QS, S),           # hB Q^T strip 3
                        (hB, 2 * S, 3 * S),        # hB V
                        (hA, 512, S),              # hA Q^T rest
                        (hA, S + 512, 2 * S),      # hA K^T rest
                        (hA, 2 * S + 512, 3 * S),  # hA V rest
                        (hB, 0, 3 * QS),           # hB Q^T rest
                    ):
                        nc.sync.dma_start(
                            out=sbufs[h][:, c0:c1], in_=qkvT[h][:, c0:c1]
                        )
                else:
                    for h, c0, c1 in (
                        (hA, S, S + 512),          # hA K^T blocks 0-3
                        (hA, 0, 512),              # hA Q^T strip 0
                        (hA, 2 * S, 2 * S + 512),  # hA V blocks 0-3
                        (hB, S, 2 * S),            # hB K^T
                        (hB, 3 * QS, S),           # hB Q^T strip 3
                        (hB, 2 * S, 3 * S),        # hB V
                        (hA, 512, S),              # hA Q^T rest
                        (hA, S + 512, 2 * S),      # hA K^T rest
                        (hA, 2 * S + 512, 3 * S),  # hA V rest
                        (hB, 0, 3 * QS),           # hB Q^T rest
                    ):
                        nc.sync.dma_start(
                            out=sbufs[h][:, c0:c1], in_=qkvT[h][:, c0:c1]
                        )

                v8s = {}
                for h in (hA, hB):
                    v8_sb = qkpool.tile([128, 1536], f8, tag="v8_sb")
                    nc.sync.dma_start(out=v8_sb[:], in_=v8_d[h])
                    v8s[h] = v8_sb
                den = psd.tile([128, QS], f32, tag="den")
                den_state = [False]
                slots = []
                for s in range(NSTRIP):
                    slots.append((hA, s))
                    slots.append((hB, NSTRIP - 1 - s))
                for si, (h, s) in enumerate(slots):
                    row = 32 * s + (16 if h % 2 else 0)
                    emit_strip(
                        h, s, sbufs[h], v8s[h], den, row, den_state,
                        si == len(slots) - 1,
                    )
                    if si == 5 and pr_i + 1 < HPC // 2:
                        # prefetch the next pair's inputs late in this pair
                        # so its first slots don't wait on DMA
                        load_pair(2 * pr_i + 2, 2 * pr_i + 3)
                    if pending:
                        emit_epilogue(pending.pop(0))

                # one batched reciprocal per head pair (rows 0,16,...,112
                # hold the 8 strips' denominators)
                recip = rcpool.tile([128, QS], f32r, tag="recip")
                nc.vector.reciprocal(recip[:], den[:])
                for h in (hA, hB):
                    for s in range(NSTRIP):
                        row = 32 * s + (16 if h % 2 else 0)
                        pending.append((h, s, row, den, recip))
            while pending:
                emit_epilogue(pending.pop(0), tail=True)
    nc.compile()
    return nc


def get_nc():
    if "nc" not in _nc_cache:
        _nc_cache["nc"] = _build_nc()
    return _nc_cache["nc"]


def _build_tri():
    dk = np.arange(128)[:, None]
    c = np.arange(128)[None, :]
    return np.where(dk <= c, 0.0, NEG).astype(np.float32)


def make_in_maps(qkv):
    import ml_dtypes

    qkv = np.asarray(qkv, dtype=np.float32)
    tri = _build_tri()
    # Denominator weights, [128, (a=2) x (m=256)] fp8: column m selects the
    # output partition of an M=128 matmul sliced at [112-r, 240-r). 1.0 at
    # m=112 (the target row r); exactly 0 at other m = 0 mod 16 (those map
    # onto other LIVE den rows); 2^-6 elsewhere so never-live rows hold a
    # finite denominator (reciprocal of 0 would be Inf, and 0*Inf = NaN in
    # the K=32 broadcast matmul).
    m = np.arange(256)
    col = np.where(m % 16 == 0, 0.0, EPS8).astype(np.float32)
    col[112] = 1.0
    ones8 = np.broadcast_to(
        np.concatenate([col, col])[None, :], (128, 512)
    ).astype(ml_dtypes.float8_e4m3)
    # K=32 selector weights for the reciprocal broadcast: partition p of
    # cols [0,128) is 1.0 iff p % 32 == 0; of cols [128,256) iff p % 32 == 16.
    p = np.arange(128)[:, None]
    selv = np.concatenate(
        [
            np.where(p % 32 == 0, 1.0, 0.0).repeat(128, axis=1),
            np.where(p % 32 == 16, 1.0, 0.0).repeat(128, axis=1),
        ],
        axis=1,
    ).astype(np.float32)
    in_maps = []
    for core in range(NCORES):
        qkvT = np.empty((HPC, 128, 3 * S), np.float16)
        for i in range(HPC):
            bh = core * HPC + i
            b, h = bh // H, bh % H
            qkvT[i, :, 0:S] = qkv[b, :, 0, h, :].T
            qkvT[i, :, S : 2 * S] = qkv[b, :, 1, h, :].T
            qkvT[i, :, 2 * S : 3 * S] = (
                qkv[b, :, 2, h, :]
                .reshape(S // KB, KB, D)
                .transpose(1, 0, 2)
                .reshape(KB, S)
            )
        v8 = np.empty((HPC, 128, 1536), ml_dtypes.float8_e4m3)
        for i in range(HPC):
            bh = core * HPC + i
            b, h = bh // H, bh % H
            vkd = qkv[b, :, 2, h, :].reshape(S // KB, KB, D)  # [16, 128, 128]
            # pair p covers blocks (2p, 2p+1); layout [128, (a=2) x (d=128)]
            v8[i] = (
                vkd[0:12]
                .reshape(6, 2, KB, D)
                .transpose(2, 0, 1, 3)
                .reshape(KB, 1536)
            )
        in_maps.append(
            {"qkvT": qkvT, "tri": tri, "ones8": ones8, "selv": selv, "v8": v8}
        )
    return in_maps


def assemble_out(results):
    out = np.empty((B, S, H, D), np.float32)
    for core in range(NCORES):
        oTc = results[core]["oT"]  # [HPC, 128, S] bf16
        for i in range(HPC):
            bh = core * HPC + i
            b, h = bh // H, bh % H
            out[b, :, h, :] = oTc[i].astype(np.float32).T
    return out


def kernel(qkv):
    from concourse.bass_utils import run_bass_kernel_spmd

    in_maps = make_in_maps(qkv)
    nc = get_nc()
    res = run_bass_kernel_spmd(nc, in_maps, list(range(NCORES)))
    return assemble_out(res.results)


# revision 31
# speedup vs baseline: 1.2082x; 1.0285x over previous
"""Causal multi-head attention (QKV-packed) on 8 Trainium2 NeuronCores.

Sharding: pure head-parallel. B*H = 32 (batch, head) pairs -> 4 per core,
zero inter-core communication. Flash-style causal attention per head, all
in the "transposed" orientation (k on partitions) so no on-device
transposes are needed:

  - Host pre-lays-out Q^T, K^T as [D=128, S] (fp16, D on partitions) and V
    as k-blocks [128, D] (fp16). Scores for a PAIR of k-blocks land in one
    [128, 1024] PSUM tile (2 banks); one ACT instruction computes
    pt = exp(scale*s - 2) over the written extent. The -2 bias keeps
    exp <= ~45 < 240 (TRN fp8e4 max) and cancels between numerator and
    denominator. Diagonal blocks pack contiguously (t3 at [512,640), t2 at
    [512,768)) so no masked-garbage columns feed the denominator.
  - pt is fp8e4 except strip 0's diagonal pairs (bf16): short softmax rows
    (q < ~100) lack the num/den error cancellation that makes fp8 safe for
    long rows.
  - O^T[d, q] += V_j.T @ pt accumulates in PSUM per 512-col q-strip
    (fp16 x fp8/bf16, 1 col/cycle), then is evacuated to SBUF bf16 at
    strip end to free the bank.
  - Two heads are processed as a PAIR with strips interleaved
    (hA.s0, hB.s3, hA.s1, hB.s2, ...): each adjacent slot-pair has a
    constant amount of matmul work, so the PE never sees a multi-us idle
    stretch (which would re-engage the HAM clock throttle to 1.2 GHz).
  - Both heads' softmax denominators share ONE PSUM bank: strip s of the
    even head accumulates on partition 32s, of the odd head on 16+32s.
    The row is selected by the weight column of an M=128 matmul (fp8
    DoubleRow pairs at 2 cols/cycle for off-diagonal work; normal-rate
    ones-column matmuls for diagonal solo regions and strip 0). Weight
    columns that map to other live rows are exactly 0; never-live rows get
    2^-6 so their denominators stay finite (a 0 would turn the batched
    reciprocal into Inf and poison the K=32 broadcast matmul with 0*Inf).
    One DVE reciprocal per head pair serves all 8 strips.
  - Normalization: K=32 selector matmul broadcasts the reciprocal row into
    the retired den bank, one DVE cast to bf16, one bf16 DVE multiply.
    Output is bf16 (host casts to fp32). Epilogues of a head pair are
    spread one-per-slot across the next pair to avoid DVE pileups.
  - Zero-input warmup matmuls (no DMA dependency) run first so the PE HAM
    clock gate opens (1.2 -> 2.4 GHz) before real data arrives.
"""

import sys

if "/opt/trn_rl_repo" not in sys.path:
    sys.path.insert(0, "/opt/trn_rl_repo")

import numpy as np

B, S, H, D = 2, 2048, 16, 128
NCORES = 8
HPC = (B * H) // NCORES  # heads per core = 4
QS = 512   # q-strip width (PSUM bank)
KB = 128   # k-block (partition dim)
NEG = -1.0e30
SCALE = 1.0 / float(np.sqrt(D))
EXP_BIAS = -2.0
NSTRIP = S // QS  # 4
EPS8 = 0.015625  # 2^-6, min normal e4m3

_nc_cache = {}


def _build_nc():
    import concourse.bass as bass  # noqa: F401
    import concourse.mybir as mybir
    from concourse import bacc
    from concourse.tile import TileContext

    f32 = mybir.dt.float32
    f16 = mybir.dt.float16
    f8 = mybir.dt.float8e4
    bf16 = mybir.dt.bfloat16
    f32r = mybir.dt.float32r
    Exp = mybir.ActivationFunctionType.Exp
    DR = mybir.MatmulPerfMode.DoubleRow

    nc = bacc.Bacc()
    # One packed input per head [128, 3*S] fp16:
    # cols [0,S) = Q^T, [S,2S) = K^T, [2S,3S) = V swizzled so column
    # block j holds the V k-block [128, D] (v[p, j*KB+d] = V[j*KB+p, d]).
    qkvT = nc.declare_dram_parameter("qkvT", [HPC, 128, 3 * S], f16, isOutput=False)
    v8_d = nc.declare_dram_parameter("v8", [HPC, 128, 1536], f8, isOutput=False)
    tri_d = nc.declare_dram_parameter("tri", [128, 128], f32, isOutput=False)
    ones8_d = nc.declare_dram_parameter("ones8", [128, 512], f8, isOutput=False)
    selv_d = nc.declare_dram_parameter("selv", [128, 256], f32r, isOutput=False)
    oT = nc.declare_dram_parameter("oT", [HPC, 128, S], bf16, isOutput=True)

    with TileContext(nc) as tc:
        with (
            nc.allow_low_precision(reason="fp16/fp8/bf16 staging is within tolerance"),
            tc.tile_pool(name="cpool", bufs=1) as cpool,
            tc.tile_pool(name="qkpool", bufs=4) as qkpool,
            tc.tile_pool(name="ptpool", bufs=8) as ptpool,
            tc.tile_pool(name="ptbpool", bufs=4) as ptbpool,
            tc.tile_pool(name="orpool", bufs=16) as orpool,
            tc.tile_pool(name="rcpool", bufs=2) as rcpool,
            tc.tile_pool(name="obpool", bufs=4) as obpool,
            tc.tile_pool(name="scp", bufs=2, space="PSUM") as scp,
            tc.tile_pool(name="pso", bufs=2, space="PSUM") as pso,
            tc.tile_pool(name="psd", bufs=2, space="PSUM") as psd,
        ):
            tri_sb = cpool.tile([128, 128], f32)
            nc.sync.dma_start(out=tri_sb[:], in_=tri_d[:])
            ones8 = cpool.tile([128, 512], f8)
            nc.sync.dma_start(out=ones8[:], in_=ones8_d[:])
            selv = cpool.tile([128, 256], f32r)
            nc.sync.dma_start(out=selv[:], in_=selv_d[:])
            biasc = cpool.tile([128, 1], f32)
            nc.gpsimd.memset(biasc[:], EXP_BIAS)
            # [128, 2, 256] pair view of the den weights (1.0 at m=112)
            o83 = ones8[:, 0:512].rearrange("p (a m) -> p a m", a=2)

            # HAM warmup: zero-input matmuls with no DMA dependency keep the
            # PE busy from the end of the preamble so the clock gate opens
            # (1.2 -> 2.4 GHz) before the first real matmul.
            zsrc = cpool.tile([128, 384], f16)
            nc.scalar.memzero(zsrc[:])
            wps = scp.tile([128, 1024], f32, tag="sc")
            for w in range(40):
                nc.tensor.matmul(
                    wps[:, 0:256],
                    lhsT=zsrc[:, 0:128],
                    rhs=zsrc[:, 128:384],
                    start=True,
                    stop=True,
                )

            def emit_epilogue(ep, tail=False):
                h, s, row, den_t, recip_t = ep
                o_raw = o_raw_of[(h, s)]
                # broadcast recip row across partitions via a K=32 selector
                # matmul; mid-kernel it lands in the retired den bank, at
                # the kernel tail in alternating (free) score-pool banks so
                # consecutive epilogues don't serialize on one bank
                wa = 32 * s
                sel = selv[wa : wa + 32, 0:128] if row % 32 == 0 else (
                    selv[wa : wa + 32, 128:256]
                )
                if tail:
                    rbt = scp.tile([128, 1024], f32, tag="sc")
                    rb = rbt[:, 0:QS]
                else:
                    rb = den_t[:, :]
                nc.tensor.matmul(
                    rb,
                    lhsT=sel,
                    rhs=recip_t[wa : wa + 32, 0:QS],
                    start=True,
                    stop=True,
                    tile_position=(wa, 0),
                )
                o_sb = obpool.tile([128, QS], bf16, tag="o_sb")
                nc.vector.tensor_mul(o_sb[:], o_raw[:], rb)
                nc.sync.dma_start(out=oT[h][:, QS * s : QS * (s + 1)], in_=o_sb[:])

            pending = []
            o_raw_of = {}
            prefetched = {}

            def load_pair(hA, hB):
                if hA in prefetched:
                    return prefetched[hA]
                sbufs = {}
                for h in (hA, hB):
                    qkv_sb = qkpool.tile([128, 3 * S], f16, tag="qkv_sb")
                    sbufs[h] = qkv_sb
                if hA == 0:
                    # interleave both heads' critical chunks so slot 1
                    # (hA strip 0) and slot 2 (hB strip 3) both start early
                    for h, c0, c1 in (
                        (hB, S, 2 * S),            # hB K^T
                        (hB, 3 * QS, S),           # hB Q^T strip 3
                        (hB, 2 * S, 3 * S),        # hB V
                        (hA, S, S + 512),          # hA K^T blocks 0-3
                        (hA, 0, 512),              # hA Q^T strip 0
                        (hA, 2 * S, 2 * S + 512),  # hA V blocks 0-3
                        (hA, 512, S),              # hA Q^T rest
                        (hA, S + 512, 2 * S),      # hA K^T rest
                        (hA, 2 * S + 512, 3 * S),  # hA V rest
                        (hB, 0, 3 * QS),           # hB Q^T rest
                    ):
                        nc.sync.dma_start(
                            out=sbufs[h][:, c0:c1], in_=qkvT[h][:, c0:c1]
                        )
                else:
                    for h, c0, c1 in (
                        (hB, S, 2 * S),            # hB K^T
                        (hB, 3 * QS, S),           # hB Q^T strip 3
                        (hB, 2 * S, 3 * S),        # hB V
                        (hA, S, S + 512),          # hA K^T blocks 0-3
                        (hA, 0, 512),              # hA Q^T strip 0
                        (hA, 2 * S, 2 * S + 512),  # hA V blocks 0-3
                        (hA, 512, S),              # hA Q^T rest
                        (hA, S + 512, 2 * S),      # hA K^T rest
                        (hA, 2 * S + 512, 3 * S),  # hA V rest
                        (hB, 0, 3 * QS),           # hB Q^T rest
                    ):
                        nc.sync.dma_start(
                            out=sbufs[h][:, c0:c1], in_=qkvT[h][:, c0:c1]
                        )
                v8s = {}
                for h in (hA, hB):
                    v8_sb = qkpool.tile([128, 1536], f8, tag="v8_sb")
                    nc.sync.dma_start(out=v8_sb[:], in_=v8_d[h])
                    v8s[h] = v8_sb
                prefetched[hA] = (sbufs, v8s)
                return prefetched[hA]

            def emit_strip(h, s, qkv_sb, v8_sb, den, row, den_state, pair_last_slot):
                """Emit one q-strip of head h. den_state = [started]."""
                qt = qkv_sb[:, 0:S]
                kt = qkv_sb[:, S : 2 * S]
                vv = qkv_sb[:, 2 * S : 3 * S]
                r = row
                o_ps = pso.tile([128, QS], f32, tag="o_ps")
                q0 = QS * s

                def den_flags(last_of_strip):
                    st = not den_state[0]
                    den_state[0] = True
                    sp = pair_last_slot and last_of_strip
                    return st, sp

                # pairs: (jA, jB, woff, wN, bcol, NB, triA, triB)
                pairs = []
                for p in range(2 * s):
                    pairs.append((2 * p, 2 * p + 1, 0, QS, 512, QS, None, None))
                t0, t1, t2, t3 = 4 * s, 4 * s + 1, 4 * s + 2, 4 * s + 3
                # X = (t0, t3): t0 covers [0,512), t3 at tile [512,640)
                pairs.append((t0, t3, 0, QS, 512, 128, 0, 384))
                # Y = (t1, t2): t1 covers [128,512) at tile [0,384),
                # t2 at tile [512,768); tile [384,512) stays stale and its
                # exp output is never read
                pairs.append((t1, t2, 128, 384, 512, 256, 128, 256))

                npair = len(pairs)
                deferred = []

                def emit_pv_den(p):
                    (pi, jA, jB, woff, wN, bcol, NB, pt, diag, s0d) = p
                    first, last = (pi == 0), (pi == npair - 1)
                    if not diag:
                        # full pair: one fp8 DoubleRow PV over both k-blocks
                        # (V fp8 is safe off-diagonal: long-row softmax is
                        # diffuse, so per-element V quantization noise
                        # averages out)
                        v8p = v8_sb[:, 256 * (jA // 2) : 256 * (jA // 2) + 256]
                        nc.tensor.matmul(
                            o_ps[:, 0:QS],
                            lhsT=v8p.rearrange("p (a d) -> p a d", a=2),
                            rhs=pt[:, 0:1024].rearrange("p (a b) -> p a b", a=2),
                            start=first,
                            stop=False,
                            perf_mode=DR,
                        )
                    else:
                        nc.tensor.matmul(
                            o_ps[:, woff : woff + wN],
                            lhsT=vv[:, KB * jA : KB * (jA + 1)],
                            rhs=pt[:, 0:wN],
                            start=first,
                            stop=False,
                        )
                        nc.tensor.matmul(
                            o_ps[:, woff + wN - NB : woff + wN],
                            lhsT=vv[:, KB * jB : KB * (jB + 1)],
                            rhs=pt[:, bcol : bcol + NB],
                            start=False,
                            stop=last,
                        )
                    if not diag:
                        # full pair: DoubleRow over the whole strip
                        st, sp = den_flags(False)
                        nc.tensor.matmul(
                            den[0:128, 0:QS],
                            lhsT=o83[:, :, 112 - r : 240 - r],
                            rhs=pt[:, 0:1024].rearrange("p (a b) -> p a b", a=2),
                            start=st,
                            stop=sp,
                            perf_mode=DR,
                        )
                    elif s0d:
                        # strip 0 diagonal (bf16 pt): two normal-rate
                        # ones-column matmuls per pair
                        st, _ = den_flags(False)
                        nc.tensor.matmul(
                            den[0:128, woff : woff + wN],
                            lhsT=ones8[:, 112 - r : 240 - r],
                            rhs=pt[:, 0:wN],
                            start=st,
                            stop=False,
                        )
                        st, sp = den_flags(last)
                        nc.tensor.matmul(
                            den[0:128, woff + wN - NB : woff + wN],
                            lhsT=ones8[:, 112 - r : 240 - r],
                            rhs=pt[:, bcol : bcol + NB],
                            start=False,
                            stop=sp,
                        )
                    else:
                        # diagonal pair, fp8: solo region (A only) at normal
                        # rate + the overlap region as a DoubleRow pair with
                        # stride (bcol - solo)
                        solo = wN - NB
                        st, _ = den_flags(False)
                        nc.tensor.matmul(
                            den[0:128, woff : woff + solo],
                            lhsT=ones8[:, 112 - r : 240 - r],
                            rhs=pt[:, 0:solo],
                            start=st,
                            stop=False,
                        )
                        st, sp = den_flags(last)
                        pr = pt[:, solo : solo + 2 * (bcol - solo)].rearrange(
                            "p (a b) -> p a b", a=2
                        )
                        nc.tensor.matmul(
                            den[0:128, woff + solo : woff + wN],
                            lhsT=o83[:, :, 112 - r : 240 - r],
                            rhs=pr[:, :, 0:NB],
                            start=False,
                            stop=sp,
                            perf_mode=DR,
                        )

                for pi, (jA, jB, woff, wN, bcol, NB, trA, trB) in enumerate(pairs):
                    diag = pi >= npair - 2
                    s0d = diag and s == 0
                    sc = scp.tile([128, 1024], f32, tag="sc")
                    nc.tensor.matmul(
                        sc[:, 0:wN],
                        lhsT=kt[:, KB * jA : KB * (jA + 1)],
                        rhs=qt[:, q0 + woff : q0 + woff + wN],
                        start=True,
                        stop=True,
                    )
                    bq = q0 + woff + wN - NB
                    nc.tensor.matmul(
                        sc[:, bcol : bcol + NB],
                        lhsT=kt[:, KB * jB : KB * (jB + 1)],
                        rhs=qt[:, bq : bq + NB],
                        start=True,
                        stop=True,
                    )
                    if trA is not None:
                        # both 128-col triangular masks in one strided DVE op
                        sc3 = sc[:, 0:1024].rearrange("p (a b) -> p a b", a=2)[
                            :, :, 0:128
                        ]
                        tri3 = tri_sb[:, 0:128].unsqueeze(1).broadcast_to(
                            [128, 2, 128]
                        )
                        nc.vector.tensor_add(sc3, sc3, tri3)
                    ext = bcol + NB
                    if s0d:
                        pt = ptbpool.tile([128, 1024], bf16, tag="ptb")
                    else:
                        pt = ptpool.tile([128, 1024], f8, tag="pt")
                    nc.scalar.activation(
                        pt[:, 0:ext], sc[:, 0:ext], Exp, bias=biasc[:], scale=SCALE
                    )
                    # defer PV/den two pairs so the PE always has score
                    # matmuls queued ahead of work that waits on ACT output
                    deferred.append((pi, jA, jB, woff, wN, bcol, NB, pt, diag, s0d))
                    if len(deferred) > 3:
                        emit_pv_den(deferred.pop(0))
                while deferred:
                    emit_pv_den(deferred.pop(0))
                # evacuate O^T early (frees the PSUM bank; bf16 is fine for
                # the un-normalized accumulator)
                o_raw = orpool.tile([128, QS], bf16, tag="o_raw")
                nc.vector.tensor_copy(o_raw[:], o_ps[:])
                o_raw_of[(h, s)] = o_raw

            for pr_i in range(HPC // 2):
                hA, hB = 2 * pr_i, 2 * pr_i + 1
                sbufs = {}
                for h in (hA, hB):
                    qkv_sb = qkpool.tile([128, 3 * S], f16, tag="qkv_sb")
                    sbufs[h] = qkv_sb
                if hA == 0:
                    # interleave both heads' critical chunks so slot 1
                    # (hA strip 0) and slot 2 (hB strip 3) both start early
                    for h, c0, c1 in (
                        (hB, S, 2 * S),            # hB K^T
                        (hB, 3 * QS, S),           # hB Q^T strip 3
                        (hB, 2 * S, 3 * S),        # hB V
                        (hA, S, S + 512),          # hA K^T blocks 0-3
                        (hA, 0, 512),              # hA Q^T strip 0
                        (hA, 2 * S, 2 * S + 512),  # hA V blocks 0-3
                        (hA, 512, S),              # hA Q^T rest
                        (hA, S + 512, 2 * S),      # hA K^T rest
                        (hA, 2 * S + 512, 3 * S),  # hA V rest
                        (hB, 0, 3 * QS),           # hB Q^T rest
                    ):
                        nc.sync.dma_start(
                            out=sbufs[h][:, c0:c1], in_=qkvT[h][:, c0:c1]
                        )
                else:
                    for h, c0, c1 in (
                        (hB, S, 2 * S),            # hB K^T
                        (hB, 3 * QS, S),           # hB Q^T strip 3
                        (hB, 2 * S, 3 * S),        # hB V
                        (hA, S, S + 512),          # hA K^T blocks 0-3
                        (hA, 0, 512),              # hA Q^T strip 0
                        (hA, 2 * S, 2 * S + 512),  # hA V blocks 0-3
                        (hA, 512, S),              # hA Q^T rest
                        (hA, S + 512, 2 * S),      # hA K^T rest
                        (hA, 2 * S + 512, 3 * S),  # hA V rest
                        (hB, 0, 3 * QS),           # hB Q^T rest
                    ):
                        nc.sync.dma_start(
                            out=sbufs[h][:, c0:c1], in_=qkvT[h][:, c0:c1]
                        )

                v8s = {}
                for h in (hA, hB):
                    v8_sb = qkpool.tile([128, 1536], f8, tag="v8_sb")
                    nc.sync.dma_start(out=v8_sb[:], in_=v8_d[h])
                    v8s[h] = v8_sb
                den = psd.tile([128, QS], f32, tag="den")
                den_state = [False]
                slots = []
                for s in range(NSTRIP):
                    # heavy strips at the pair boundaries: s3 first and last
                    slots.append((hB, NSTRIP - 1 - s))
                    slots.append((hA, s))
                for si, (h, s) in enumerate(slots):
                    row = 32 * s + (16 if h % 2 else 0)
                    emit_strip(
                        h, s, sbufs[h], v8s[h], den, row, den_state,
                        si == len(slots) - 1,
                    )
                    if si == 5 and pr_i + 1 < HPC // 2:
                        # prefetch the next pair's inputs late in this pair
                        # so its first slots don't wait on DMA
                        load_pair(2 * pr_i + 2, 2 * pr_i + 3)
                    if pending:
                        emit_epilogue(pending.pop(0))

                # one batched reciprocal per head pair (rows 0,16,...,112
                # hold the 8 strips' denominators)
                recip = rcpool.tile([128, QS], f32r, tag="recip")
                nc.vector.reciprocal(recip[:], den[:])
                for h in (hA, hB):
                    for s in range(NSTRIP):
                        row = 32 * s + (16 if h % 2 else 0)
                        pending.append((h, s, row, den, recip))
            while pending:
                emit_epilogue(pending.pop(0), tail=True)
    nc.compile()
    return nc


def get_nc():
    if "nc" not in _nc_cache:
        _nc_cache["nc"] = _build_nc()
    return _nc_cache["nc"]


def _build_tri():
    dk = np.arange(128)[:, None]
    c = np.arange(128)[None, :]
    return np.where(dk <= c, 0.0, NEG).astype(np.float32)


def make_in_maps(qkv):
    import ml_dtypes

    qkv = np.asarray(qkv, dtype=np.float32)
    tri = _build_tri()
    # Denominator weights, [128, (a=2) x (m=256)] fp8: column m selects the
    # output partition of an M=128 matmul sliced at [112-r, 240-r). 1.0 at
    # m=112 (the target row r); exactly 0 at other m = 0 mod 16 (those map
    # onto other LIVE den rows); 2^-6 elsewhere so never-live rows hold a
    # finite denominator (reciprocal of 0 would be Inf, and 0*Inf = NaN in
    # the K=32 broadcast matmul).
    m = np.arange(256)
    col = np.where(m % 16 == 0, 0.0, EPS8).astype(np.float32)
    col[112] = 1.0
    ones8 = np.broadcast_to(
        np.concatenate([col, col])[None, :], (128, 512)
    ).astype(ml_dtypes.float8_e4m3)
    # K=32 selector weights for the reciprocal broadcast: partition p of
    # cols [0,128) is 1.0 iff p % 32 == 0; of cols [128,256) iff p % 32 == 16.
    p = np.arange(128)[:, None]
    selv = np.concatenate(
        [
            np.where(p % 32 == 0, 1.0, 0.0).repeat(128, axis=1),
            np.where(p % 32 == 16, 1.0, 0.0).repeat(128, axis=1),
        ],
        axis=1,
    ).astype(np.float32)
    in_maps = []
    for core in range(NCORES):
        qkvT = np.empty((HPC, 128, 3 * S), np.float16)
        for i in range(HPC):
            bh = core * HPC + i
            b, h = bh // H, bh % H
            qkvT[i, :, 0:S] = qkv[b, :, 0, h, :].T
            qkvT[i, :, S : 2 * S] = qkv[b, :, 1, h, :].T
            qkvT[i, :, 2 * S : 3 * S] = (
                qkv[b, :, 2, h, :]
                .reshape(S // KB, KB, D)
                .transpose(1, 0, 2)
                .reshape(KB, S)
            )
        v8 = np.empty((HPC, 128, 1536), ml_dtypes.float8_e4m3)
        for i in range(HPC):
            bh = core * HPC + i
            b, h = bh // H, bh % H
            vkd = qkv[b, :, 2, h, :].reshape(S // KB, KB, D)  # [16, 128, 128]
            # pair p covers blocks (2p, 2p+1); layout [128, (a=2) x (d=128)]
            v8[i] = (
                vkd[0:12]
                .reshape(6, 2, KB, D)
                .transpose(2, 0, 1, 3)
                .reshape(KB, 1536)
            )
        in_maps.append(
            {"qkvT": qkvT, "tri": tri, "ones8": ones8, "selv": selv, "v8": v8}
        )
    return in_maps


def assemble_out(results):
    out = np.empty((B, S, H, D), np.float32)
    for core in range(NCORES):
        oTc = results[core]["oT"]  # [HPC, 128, S] bf16
        for i in range(HPC):
            bh = core * HPC + i
            b, h = bh // H, bh % H
            out[b, :, h, :] = oTc[i].astype(np.float32).T
    return out


def kernel(qkv):
    from concourse.bass_utils import run_bass_kernel_spmd

    in_maps = make_in_maps(qkv)
    nc = get_nc()
    res = run_bass_kernel_spmd(nc, in_maps, list(range(NCORES)))
    return assemble_out(res.results)
